# revision 20
# baseline (speedup 1.0000x reference)
"""Trainium2 Bass kernel: batched dense attention (softmax(Q S^T / sqrt(H)) S).

Full problem: query [4, 4096, 1024], source [4, 4096, 1024] (source doubles
as values), output [4, 4096, 1024], all float32.

Sharding: pure data parallel over 8 NeuronCores — core c handles batch
c//2, query rows (c%2)*2048 ... +2048 with the full source for that batch
replicated to the core host-side.  No collectives are needed.

The host pre-casts Q and S to bf16 (the on-chip compute dtype), halving
staging DMA and removing all on-chip casts.

Per-core kernel (transposed-P1 formulation, bf16 matmuls, f32 PSUM).  The
PE does pure matmul work — no PE transposes at all:
  - S^T staged by 8 DRAM->SBUF xbar DMA-transposes; Q^T by one xbar
    DMA-transpose per 512-query block.  All xbar transposes ride the same
    (ACT) HWDGE ring — the xbar destination base is engine register state,
    so transposes on different rings corrupt each other.
  - S natural layout staged with a leading all-ones column per source
    tile ([128, 1+h] each), so P2 computes softmax row sums for free.
  - per 512-query block:
      P1T: logitsT tiles [128s, 512q] = S^T_tile.T @ Q^T  (contract over H)
      exp on ACT with scale=1/32 -> W^T tiles [128s, 512q] in SBUF, which
      are directly P2's stationary operand (this is why P1 is transposed:
      no W transpose is ever needed, and no softmax-sum accumulation).
      P2 per 128-query tile: O'[q, {R, h}] += W^T.T @ [1|S] over 3 PSUM
      chunks (342+342+341 cols); chunk 0 col 0 is the row sum R.
      Normalize by 1/R on DVE while writing bf16 output.
"""

import math

import numpy as np

B, LQ, LS, H = 4, 4096, 4096, 1024
N_CORES = 8
Q_SPLIT = 2  # query-length split within each batch entry
LQ_SH = LQ // Q_SPLIT  # 2048 query rows per core

P = 128  # partitions
QB = 512  # query-block width (P1T moving columns)
SC = 512  # S^T staging chunk width (s columns per xbar transpose)
N_WARM = 13  # dummy matmuls to promote the PE HAM clock gate at t=0
# P2 output chunking: 1 (ones) + 1024 (h) columns in 3 PSUM chunks
P2_CHUNKS = ((0, 342), (342, 342), (684, 341))


def _build(lq_sh, ls, h):
    """Build + compile the per-core Bass graph for shard shapes."""
    import concourse.bacc as bacc
    import concourse.mybir as mybir
    import concourse.tile as tile

    f32 = mybir.dt.float32
    bf16 = mybir.dt.bfloat16

    n_st = ls // P  # source tiles
    n_hc = h // P  # h chunks (contraction tiles for P1T)
    sc = min(SC, ls)
    n_sc = ls // sc  # S^T staging chunks
    st_per_sc = sc // P
    qb = min(QB, lq_sh)
    n_qb = lq_sh // qb  # query blocks
    qt_per_qb = qb // P
    w = h + 1  # per-source-tile staged width (ones column + h)
    scale = 1.0 / math.sqrt(h)

    nc = bacc.Bacc(
        "TRN2",
        target_bir_lowering=False,
        debug=False,
        num_devices=N_CORES,
    )
    qt_h = nc.dram_tensor("q_t", [h, lq_sh], bf16, kind="ExternalInput")
    s_h = nc.dram_tensor("source_input", [ls, h], bf16, kind="ExternalInput")
    st_h = nc.dram_tensor("s_t", [h, ls], bf16, kind="ExternalInput")
    o_h = nc.dram_tensor("out", [lq_sh, h], bf16, kind="ExternalOutput")
    s_ap, o_ap = s_h.ap(), o_h.ap()
    # [h, n] DRAM views as [p, hc, n]: row hc*P + p
    qt_ap3 = qt_h.ap().rearrange("(hc p) n -> p hc n", p=P)
    st_ap3 = st_h.ap().rearrange("(hc p) n -> p hc n", p=P)

    with tile.TileContext(nc) as tc:
        from contextlib import ExitStack

        with ExitStack() as ctx:
            # PE clock warmup: a dense burst of junk matmuls at t=0 fills the
            # HAM activity window so the 2.4 GHz clock engages before real
            # matmul work arrives (and mainline density then keeps it warm).
            warm_pool = ctx.enter_context(tc.tile_pool(name="warm", bufs=1))
            warm_w = warm_pool.tile([P, P], bf16)
            warm_x = warm_pool.tile([P, qb], bf16)
            nc.vector.memset(warm_w[:], 0.0)
            nc.vector.memset(warm_x[:], 0.0)
            psum_lg = ctx.enter_context(
                tc.tile_pool(name="psum_lg", bufs=4, space="PSUM")
            )
            wp = psum_lg.tile([P, qb], f32, tag="lgT", name="warmpsum")
            for _ in range(N_WARM):
                nc.tensor.matmul(wp[:], warm_w[:], warm_x[:], start=True, stop=True)

            qT_pool = ctx.enter_context(tc.tile_pool(name="qT", bufs=2))
            qTs = {}

            def issue_qT(b):
                # qT[p, hc, j] = Q^T[hc*P + p, b*qb + j], plain DMA
                t = qT_pool.tile([P, n_hc * qb], bf16, tag="qT")
                nc.sync.dma_start(
                    t.rearrange("p (hc j) -> p hc j", j=qb),
                    qt_ap3[:, :, b * qb : (b + 1) * qb],
                )
                qTs[b] = t

            issue_qT(0)

            persist = ctx.enter_context(tc.tile_pool(name="persist", bufs=1))
            # S^T in n_sc chunks; chunk sci holds h-chunk hc at cols
            # [hc*sc, +sc): sT[p, hc*sc + j] = S[sci*sc + j, hc*P + p].
            # Chunk 0 is staged as st_per_sc per-source-tile tiles instead, so
            # the first P1T chain's stationary (256 KB) lands ~4us earlier
            # than a whole 1 MiB chunk would.
            s_T2 = [
                persist.tile([P, n_hc * P], bf16, tag=f"sT2_{i}", name=f"sT2_{i}")
                for i in range(st_per_sc)
            ]
            s_T = {
                i: persist.tile([P, n_hc * sc], bf16, tag=f"sT{i}", name=f"sT{i}")
                for i in range(1, n_sc)
            }
            for st in range(st_per_sc):
                nc.sync.dma_start(
                    s_T2[st].rearrange("p (hc j) -> p hc j", j=P),
                    st_ap3[:, :, st * P : (st + 1) * P],
                )
            for sci in range(1, n_sc):
                nc.sync.dma_start(
                    s_T[sci].rearrange("p (hc j) -> p hc j", j=sc),
                    st_ap3[:, :, sci * sc : (sci + 1) * sc],
                )

            # S natural layout with a leading ones column per source tile:
            # tile st at cols [st*w, +w): col 0 = 1.0, cols 1..h = S[st*P+p, :].
            s_nat = persist.tile([P, n_st * w], bf16)
            nc.vector.memset(
                s_nat.rearrange("p (st c) -> p st c", c=w)[:, :, 0:1], 1.0
            )
            for st in range(n_st):
                nc.sync.dma_start(
                    s_nat[:, st * w + 1 : (st + 1) * w],
                    s_ap[st * P : (st + 1) * P, :],
                )

            # W^T tiles: one [128s, qb] tile per source tile, written by ACT
            # exp directly (the transposed-P1 trick).  Single-buffered: the
            # PE's own P2(k) -> P1T(k+1) ordering provides the reuse window.
            wT_pool = ctx.enter_context(tc.tile_pool(name="wT", bufs=1))
            psum_o = ctx.enter_context(
                tc.tile_pool(name="psum_o", bufs=4, space="PSUM")
            )
            r_pool = ctx.enter_context(tc.tile_pool(name="r", bufs=8))
            osb_pool = ctx.enter_context(tc.tile_pool(name="osb", bufs=3))

            for b in range(n_qb):
                if b + 1 < n_qb:
                    issue_qT(b + 1)
                qT = qTs.pop(b)
                wT = [
                    wT_pool.tile([P, qb], bf16, tag=f"wt{st}", name=f"wt{st}")
                    for st in range(n_st)
                ]
                # P1T: logitsT tiles, one per source tile
                for st in range(n_st):
                    sci, soff = divmod(st, st_per_sc)
                    lgT = psum_lg.tile([P, qb], f32, tag="lgT")
                    for hc in range(n_hc):
                        if sci == 0:
                            stat = s_T2[st][:, hc * P : (hc + 1) * P]
                        else:
                            stat = s_T[sci][
                                :, hc * sc + soff * P : hc * sc + (soff + 1) * P
                            ]
                        nc.tensor.matmul(
                            lgT[:],
                            stat,
                            qT[:, hc * qb : (hc + 1) * qb],
                            start=(hc == 0),
                            stop=(hc == n_hc - 1),
                        )
                    nc.scalar.activation(
                        wT[st][:],
                        lgT[:],
                        mybir.ActivationFunctionType.Exp,
                        scale=scale,
                    )

                # P2 per 128-query tile: 3 chunks over [1|S]; chunk 0 col 0
                # accumulates the softmax row sum.
                for qs in range(qt_per_qb):
                    ob = osb_pool.tile([P, h], bf16, tag="ob")
                    rinv = r_pool.tile([P, 1], f32, tag="rinv")
                    for ci, (coff, cw) in enumerate(P2_CHUNKS):
                        opt = psum_o.tile([P, P2_CHUNKS[0][1]], f32, tag="op", name="op")
                        op = opt[:, :cw]
                        for st in range(n_st):
                            nc.tensor.matmul(
                                op,
                                wT[st][:, qs * P : (qs + 1) * P],
                                s_nat[:, st * w + coff : st * w + coff + cw],
                                start=(st == 0),
                                stop=(st == n_st - 1),
                            )
                        if ci == 0:
                            nc.vector.reciprocal(rinv[:], op[:, 0:1])
                            nc.vector.tensor_scalar_mul(
                                ob[:, 0 : cw - 1], op[:, 1:cw], rinv[:]
                            )
                        else:
                            nc.vector.tensor_scalar_mul(
                                ob[:, coff - 1 : coff - 1 + cw], op, rinv[:]
                            )
                        # store each chunk as soon as it is normalized: the
                        # final tile drains during the last chunks' matmuls
                        lo = 0 if ci == 0 else coff - 1
                        hi = coff - 1 + cw
                        qrow = (b * qt_per_qb + qs) * P
                        nc.sync.dma_start(
                            o_ap[qrow : qrow + P, lo:hi], ob[:, lo:hi]
                        )

    nc.compile()
    return nc


_cached_nc = None


def _get_nc():
    global _cached_nc
    if _cached_nc is None:
        _cached_nc = _build(LQ_SH, LS, H)
    return _cached_nc


def _in_maps(query_input, source_input):
    import ml_dtypes

    bf16 = ml_dtypes.bfloat16
    q = np.asarray(query_input, dtype=np.float32).astype(bf16)
    s = np.asarray(source_input, dtype=np.float32).astype(bf16)
    assert q.shape == (B, LQ, H) and s.shape == (B, LS, H)
    in_maps = []
    for c in range(N_CORES):
        b, qh = divmod(c, Q_SPLIT)
        in_maps.append(
            {
                "q_t": np.ascontiguousarray(
                    q[b, qh * LQ_SH : (qh + 1) * LQ_SH, :].T
                ),
                "source_input": np.ascontiguousarray(s[b]),
                "s_t": np.ascontiguousarray(s[b].T),
            }
        )
    return in_maps


def _gather(results):
    out = np.empty((B, LQ, H), dtype=np.float32)
    for c in range(N_CORES):
        b, qh = divmod(c, Q_SPLIT)
        out[b, qh * LQ_SH : (qh + 1) * LQ_SH, :] = results[c]["out"]
    return out


def kernel(query_input, source_input):
    from concourse.bass_utils import run_bass_kernel_spmd

    res = run_bass_kernel_spmd(
        _get_nc(),
        _in_maps(query_input, source_input),
        core_ids=list(range(N_CORES)),
    )
    return _gather(res.results)


# revision 21
# speedup vs baseline: 1.0006x; 1.0006x over previous
"""Trainium2 Bass kernel: batched dense attention (softmax(Q S^T / sqrt(H)) S).

Full problem: query [4, 4096, 1024], source [4, 4096, 1024] (source doubles
as values), output [4, 4096, 1024], all float32.

Sharding: pure data parallel over 8 NeuronCores — core c handles batch
c//2, query rows (c%2)*2048 ... +2048 with the full source for that batch
replicated to the core host-side.  No collectives are needed.

The host pre-casts Q and S to bf16 (the on-chip compute dtype), halving
staging DMA and removing all on-chip casts.

Per-core kernel (transposed-P1 formulation, bf16 matmuls, f32 PSUM).  The
PE does pure matmul work — no PE transposes at all:
  - S^T staged by 8 DRAM->SBUF xbar DMA-transposes; Q^T by one xbar
    DMA-transpose per 512-query block.  All xbar transposes ride the same
    (ACT) HWDGE ring — the xbar destination base is engine register state,
    so transposes on different rings corrupt each other.
  - S natural layout staged with a leading all-ones column per source
    tile ([128, 1+h] each), so P2 computes softmax row sums for free.
  - per 512-query block:
      P1T: logitsT tiles [128s, 512q] = S^T_tile.T @ Q^T  (contract over H)
      exp on ACT with scale=1/32 -> W^T tiles [128s, 512q] in SBUF, which
      are directly P2's stationary operand (this is why P1 is transposed:
      no W transpose is ever needed, and no softmax-sum accumulation).
      P2 per 128-query tile: O'[q, {R, h}] += W^T.T @ [1|S] over 3 PSUM
      chunks (342+342+341 cols); chunk 0 col 0 is the row sum R.
      Normalize by 1/R on DVE while writing bf16 output.
"""

import math

import numpy as np

B, LQ, LS, H = 4, 4096, 4096, 1024
N_CORES = 8
Q_SPLIT = 2  # query-length split within each batch entry
LQ_SH = LQ // Q_SPLIT  # 2048 query rows per core

P = 128  # partitions
QB = 512  # query-block width (P1T moving columns)
SC = 512  # S^T staging chunk width (s columns per xbar transpose)
N_WARM = 16  # dummy matmuls to promote the PE HAM clock gate at t=0
# P2 output chunking: 1 (ones) + 1024 (h) columns in 3 PSUM chunks
P2_CHUNKS = ((0, 342), (342, 342), (684, 341))


def _build(lq_sh, ls, h):
    """Build + compile the per-core Bass graph for shard shapes."""
    import concourse.bacc as bacc
    import concourse.mybir as mybir
    import concourse.tile as tile

    f32 = mybir.dt.float32
    bf16 = mybir.dt.bfloat16

    n_st = ls // P  # source tiles
    n_hc = h // P  # h chunks (contraction tiles for P1T)
    sc = min(SC, ls)
    n_sc = ls // sc  # S^T staging chunks
    st_per_sc = sc // P
    qb = min(QB, lq_sh)
    n_qb = lq_sh // qb  # query blocks
    qt_per_qb = qb // P
    w = h + 1  # per-source-tile staged width (ones column + h)
    scale = 1.0 / math.sqrt(h)

    nc = bacc.Bacc(
        "TRN2",
        target_bir_lowering=False,
        debug=False,
        num_devices=N_CORES,
    )
    qt_h = nc.dram_tensor("q_t", [h, lq_sh], bf16, kind="ExternalInput")
    s_h = nc.dram_tensor("source_input", [ls, h], bf16, kind="ExternalInput")
    st_h = nc.dram_tensor("s_t", [h, ls], bf16, kind="ExternalInput")
    o_h = nc.dram_tensor("out", [lq_sh, h], bf16, kind="ExternalOutput")
    s_ap, o_ap = s_h.ap(), o_h.ap()
    # [h, n] DRAM views as [p, hc, n]: row hc*P + p
    qt_ap3 = qt_h.ap().rearrange("(hc p) n -> p hc n", p=P)
    st_ap3 = st_h.ap().rearrange("(hc p) n -> p hc n", p=P)

    with tile.TileContext(nc) as tc:
        from contextlib import ExitStack

        with ExitStack() as ctx:
            # PE clock warmup: a dense burst of junk matmuls at t=0 fills the
            # HAM activity window so the 2.4 GHz clock engages before real
            # matmul work arrives (and mainline density then keeps it warm).
            warm_pool = ctx.enter_context(tc.tile_pool(name="warm", bufs=1))
            warm_w = warm_pool.tile([P, P], bf16)
            warm_x = warm_pool.tile([P, qb], bf16)
            nc.vector.memset(warm_w[:], 0.0)
            nc.vector.memset(warm_x[:], 0.0)
            psum_lg = ctx.enter_context(
                tc.tile_pool(name="psum_lg", bufs=4, space="PSUM")
            )
            wp = psum_lg.tile([P, qb], f32, tag="lgT", name="warmpsum")
            for _ in range(N_WARM):
                nc.tensor.matmul(wp[:], warm_w[:], warm_x[:], start=True, stop=True)

            qT_pool = ctx.enter_context(tc.tile_pool(name="qT", bufs=2))
            qTs = {}

            def issue_qT(b):
                # qT[p, hc, j] = Q^T[hc*P + p, b*qb + j], plain DMA
                t = qT_pool.tile([P, n_hc * qb], bf16, tag="qT")
                nc.sync.dma_start(
                    t.rearrange("p (hc j) -> p hc j", j=qb),
                    qt_ap3[:, :, b * qb : (b + 1) * qb],
                )
                qTs[b] = t

            issue_qT(0)

            persist = ctx.enter_context(tc.tile_pool(name="persist", bufs=1))
            # S^T in n_sc chunks; chunk sci holds h-chunk hc at cols
            # [hc*sc, +sc): sT[p, hc*sc + j] = S[sci*sc + j, hc*P + p].
            # Chunk 0 is staged as st_per_sc per-source-tile tiles instead, so
            # the first P1T chain's stationary (256 KB) lands ~4us earlier
            # than a whole 1 MiB chunk would.
            s_T2 = [
                persist.tile([P, n_hc * P], bf16, tag=f"sT2_{i}", name=f"sT2_{i}")
                for i in range(st_per_sc)
            ]
            s_T = {
                i: persist.tile([P, n_hc * sc], bf16, tag=f"sT{i}", name=f"sT{i}")
                for i in range(1, n_sc)
            }
            for st in range(st_per_sc):
                nc.sync.dma_start(
                    s_T2[st].rearrange("p (hc j) -> p hc j", j=P),
                    st_ap3[:, :, st * P : (st + 1) * P],
                )
            for sci in range(1, n_sc):
                nc.sync.dma_start(
                    s_T[sci].rearrange("p (hc j) -> p hc j", j=sc),
                    st_ap3[:, :, sci * sc : (sci + 1) * sc],
                )

            # S natural layout with a leading ones column per source tile:
            # tile st at cols [st*w, +w): col 0 = 1.0, cols 1..h = S[st*P+p, :].
            s_nat = persist.tile([P, n_st * w], bf16)
            nc.vector.memset(
                s_nat.rearrange("p (st c) -> p st c", c=w)[:, :, 0:1], 1.0
            )
            for st in range(n_st):
                nc.sync.dma_start(
                    s_nat[:, st * w + 1 : (st + 1) * w],
                    s_ap[st * P : (st + 1) * P, :],
                )

            # W^T tiles: one [128s, qb] tile per source tile, written by ACT
            # exp directly (the transposed-P1 trick).  Single-buffered: the
            # PE's own P2(k) -> P1T(k+1) ordering provides the reuse window.
            wT_pool = ctx.enter_context(tc.tile_pool(name="wT", bufs=1))
            psum_o = ctx.enter_context(
                tc.tile_pool(name="psum_o", bufs=4, space="PSUM")
            )
            r_pool = ctx.enter_context(tc.tile_pool(name="r", bufs=8))
            osb_pool = ctx.enter_context(tc.tile_pool(name="osb", bufs=3))

            for b in range(n_qb):
                if b + 1 < n_qb:
                    issue_qT(b + 1)
                qT = qTs.pop(b)
                wT = [
                    wT_pool.tile([P, qb], bf16, tag=f"wt{st}", name=f"wt{st}")
                    for st in range(n_st)
                ]
                # P1T: logitsT tiles, one per source tile
                for st in range(n_st):
                    sci, soff = divmod(st, st_per_sc)
                    lgT = psum_lg.tile([P, qb], f32, tag="lgT")
                    for hc in range(n_hc):
                        if sci == 0:
                            stat = s_T2[st][:, hc * P : (hc + 1) * P]
                        else:
                            stat = s_T[sci][
                                :, hc * sc + soff * P : hc * sc + (soff + 1) * P
                            ]
                        nc.tensor.matmul(
                            lgT[:],
                            stat,
                            qT[:, hc * qb : (hc + 1) * qb],
                            start=(hc == 0),
                            stop=(hc == n_hc - 1),
                        )
                    nc.scalar.activation(
                        wT[st][:],
                        lgT[:],
                        mybir.ActivationFunctionType.Exp,
                        scale=scale,
                    )

                # P2 per 128-query tile: 3 chunks over [1|S]; chunk 0 col 0
                # accumulates the softmax row sum.
                for qs in range(qt_per_qb):
                    ob = osb_pool.tile([P, h], bf16, tag="ob")
                    rinv = r_pool.tile([P, 1], f32, tag="rinv")
                    for ci, (coff, cw) in enumerate(P2_CHUNKS):
                        opt = psum_o.tile([P, P2_CHUNKS[0][1]], f32, tag="op", name="op")
                        op = opt[:, :cw]
                        for st in range(n_st):
                            nc.tensor.matmul(
                                op,
                                wT[st][:, qs * P : (qs + 1) * P],
                                s_nat[:, st * w + coff : st * w + coff + cw],
                                start=(st == 0),
                                stop=(st == n_st - 1),
                            )
                        if ci == 0:
                            nc.vector.reciprocal(rinv[:], op[:, 0:1])
                            nc.vector.tensor_scalar_mul(
                                ob[:, 0 : cw - 1], op[:, 1:cw], rinv[:]
                            )
                        else:
                            nc.vector.tensor_scalar_mul(
                                ob[:, coff - 1 : coff - 1 + cw], op, rinv[:]
                            )
                        # store each chunk as soon as it is normalized: the
                        # final tile drains during the last chunks' matmuls
                        lo = 0 if ci == 0 else coff - 1
                        hi = coff - 1 + cw
                        qrow = (b * qt_per_qb + qs) * P
                        nc.sync.dma_start(
                            o_ap[qrow : qrow + P, lo:hi], ob[:, lo:hi]
                        )

    nc.compile()
    return nc


_cached_nc = None


def _get_nc():
    global _cached_nc
    if _cached_nc is None:
        _cached_nc = _build(LQ_SH, LS, H)
    return _cached_nc


def _in_maps(query_input, source_input):
    import ml_dtypes

    bf16 = ml_dtypes.bfloat16
    q = np.asarray(query_input, dtype=np.float32).astype(bf16)
    s = np.asarray(source_input, dtype=np.float32).astype(bf16)
    assert q.shape == (B, LQ, H) and s.shape == (B, LS, H)
    in_maps = []
    for c in range(N_CORES):
        b, qh = divmod(c, Q_SPLIT)
        in_maps.append(
            {
                "q_t": np.ascontiguousarray(
                    q[b, qh * LQ_SH : (qh + 1) * LQ_SH, :].T
                ),
                "source_input": np.ascontiguousarray(s[b]),
                "s_t": np.ascontiguousarray(s[b].T),
            }
        )
    return in_maps


def _gather(results):
    out = np.empty((B, LQ, H), dtype=np.float32)
    for c in range(N_CORES):
        b, qh = divmod(c, Q_SPLIT)
        out[b, qh * LQ_SH : (qh + 1) * LQ_SH, :] = results[c]["out"]
    return out


def kernel(query_input, source_input):
    from concourse.bass_utils import run_bass_kernel_spmd

    res = run_bass_kernel_spmd(
        _get_nc(),
        _in_maps(query_input, source_input),
        core_ids=list(range(N_CORES)),
    )
    return _gather(res.results)
